# revision 36
# baseline (speedup 1.0000x reference)
"""Trainium2 Bass kernel: multi-head self-attention block (dense transformer).

Computes y = softmax((x @ Wq) (x @ Wk)^T / sqrt(H)) (x @ Wv) @ Wout + bias
for B=2, T=2048, C=1024, H=16 heads of dim 64, fp32 I/O.

Sharding: tensor-parallel over heads. Each of the 8 NeuronCores computes the
QKV projections, attention, and output projection for 2 of the 16 heads
(both batches), producing a partial output y_c = attn_out_c @ Wout[rows_c].
The host gather sums the 8 partials and adds the output bias (the
"all-reduce after the output projection" of the TP scheme, folded into the
unshard step).

Compute dtype is bf16 on the TensorEngine (fp32 PSUM accumulation, fp32
softmax statistics); inputs are rounded to bf16 on the host.

Per-batch dataflow:
  - x^T tiles (host-pretransposed, [e,t] layout) stream in; QKV projections
    run as bf16 matmuls accumulating over 8 e-tiles -> Q^T,K^T,V^T [f,t].
  - V^T is re-transposed on the PE to token-major V, stored with a ones
    column appended so the attn@V matmul also produces softmax denominators.
  - scores^T[k,q] = (K^T tile).T @ Q^T chunk over 1024-wide q-chunks with
    bf16 PSUM outputs; the two heads' K=64 matmuls go to disjoint row
    groups of the PE array and stream concurrently. exp on the scalar
    engine (scale=1/sqrt(H) folded in; no max-subtraction: |scores/4|
    stays far inside exp/fp32 range); U^T[d|sum, q] accumulates
    exp @ [V|1] over k-tiles in PSUM (two 512-wide halves).
  - normalize: 1/sum via fast DVE reciprocal, partition-broadcast by a
    DRAM bounce DMA, multiply on the vector engine -> attn_out^T.
  - projection: (attn_out^T tile).T @ Wout[rows_c] -> partial y tile.

The attention phase is scalar-engine bound (exp) while QKV/projection are
TensorEngine bound, so work is software-pipelined at three levels: the
attn@V matmuls trail the exp by one k-tile; batch b+1's QKV/transpose
matmuls fill batch b's attention iterations; and finished q-chunks'
projection matmuls are fed back into the remaining attention iterations.
"""

import math
import sys
from collections import deque
from contextlib import ExitStack

for _p in ("/opt/trn_rl_repo",):
    if _p not in sys.path:
        sys.path.insert(0, _p)

import ml_dtypes
import numpy as np

import concourse.bass as bass
import concourse.tile as tile
from concourse import bacc, mybir
from concourse.masks import make_identity
from concourse.bass_utils import run_bass_kernel_spmd

B, T, C = 2, 2048, 1024
H, D = 16, 64
NCORES = 8
HPC = H // NCORES            # heads per core = 2
FPC = HPC * D                # per-core q/k/v feature slice = 128
TQ = 512                     # PSUM fp32 free-dim tile
TQ2 = 1024                   # scores q-chunk (bf16 PSUM)
NQC = T // TQ                # 4
NQ2 = T // TQ2               # 2
NKT = T // 128               # 16 k-tiles
NET = C // 128               # 8 embedding tiles
SCALE = 1.0 / math.sqrt(H)   # NOTE: reference scales by 1/sqrt(n_head)

F32 = mybir.dt.float32
BF16 = mybir.dt.bfloat16
NPBF16 = ml_dtypes.bfloat16
AF = mybir.ActivationFunctionType
PIPELINE = True


def build_nc():
    nc = bacc.Bacc(None, target_bir_lowering=False)

    xT = nc.declare_dram_parameter("xT", [B, C, T], BF16, isOutput=False)
    wq = nc.declare_dram_parameter("wq", [C, FPC], BF16, isOutput=False)
    wk = nc.declare_dram_parameter("wk", [C, FPC], BF16, isOutput=False)
    wv = nc.declare_dram_parameter("wv", [C, FPC], BF16, isOutput=False)
    bq = nc.declare_dram_parameter("bq", [FPC, 1], F32, isOutput=False)
    bk = nc.declare_dram_parameter("bk", [FPC, 1], F32, isOutput=False)
    bv = nc.declare_dram_parameter("bv", [FPC, 1], F32, isOutput=False)
    wo = nc.declare_dram_parameter("wo", [FPC, C], BF16, isOutput=False)
    y = nc.declare_dram_parameter("y", [B, T, C], F32, isOutput=True)

    with ExitStack() as ctx:
        tc = ctx.enter_context(tile.TileContext(nc))
        consts = ctx.enter_context(tc.tile_pool(name="consts", bufs=1))
        xtp = ctx.enter_context(tc.tile_pool(name="xtp", bufs=40))
        qkvp = ctx.enter_context(tc.tile_pool(name="qkvp", bufs=4))
        vtp = ctx.enter_context(tc.tile_pool(name="vtp", bufs=2))
        vsp = ctx.enter_context(tc.tile_pool(name="vsp", bufs=2))
        expp = ctx.enter_context(tc.tile_pool(name="expp", bufs=8))
        aop = ctx.enter_context(tc.tile_pool(name="aop", bufs=2))
        outp = ctx.enter_context(tc.tile_pool(name="outp", bufs=4))
        smallp = ctx.enter_context(tc.tile_pool(name="smallp", bufs=6))
        psum = ctx.enter_context(tc.tile_pool(name="psum", bufs=2, space="PSUM"))
        dramp = ctx.enter_context(tc.tile_pool(name="dramp", bufs=8, space="DRAM"))

        ident = consts.tile([128, 128], BF16)
        make_identity(nc, ident)
        onecol_f = consts.tile([128, 1], F32)
        nc.vector.memset(onecol_f, 1.0)
        ones_row = consts.tile([1, D], F32)
        nc.vector.memset(ones_row, 1.0)

        wt_q = consts.tile([128, NET, FPC], BF16)
        nc.sync.dma_start(out=wt_q, in_=wq.rearrange("(e p) f -> p e f", p=128))
        wt_k = consts.tile([128, NET, FPC], BF16)
        wt_v = consts.tile([128, NET, FPC], BF16)
        wt_fi = (wt_q, wt_k, wt_v)

        def load_xt(b, defer_rest=False):
            # first q-chunk as its own small tile per e so the first QKV
            # chains start after 1/4 of the batch's x^T has landed
            first, rest = [], []
            for e in range(NET):
                xt0 = xtp.tile([128, TQ], BF16, tag="xt0", bufs=16, name=f"xt{b}_{e}_0")
                nc.sync.dma_start(
                    out=xt0, in_=xT[b, e * 128:(e + 1) * 128, 0:TQ]
                )
                first.append(xt0)

            def load_rest():
                r1, r23 = [], []
                for e in range(NET):
                    xt1 = xtp.tile([128, TQ], BF16, tag="xt1", bufs=16, name=f"xt{b}_{e}_1")
                    nc.sync.dma_start(
                        out=xt1, in_=xT[b, e * 128:(e + 1) * 128, TQ:2 * TQ]
                    )
                    r1.append(xt1)
                for e in range(NET):
                    xt23 = xtp.tile([128, 2 * TQ], BF16, tag="xt23", bufs=16, name=f"xt{b}_{e}_23")
                    nc.gpsimd.dma_start(
                        out=xt23, in_=xT[b, e * 128:(e + 1) * 128, 2 * TQ:T]
                    )
                    r23.append(xt23)
                rest.append((r1, r23))

            if not defer_rest:
                load_rest()

            def get(e, qc):
                if qc == 0:
                    return first[e]
                r1, r23 = rest[0]
                if qc == 1:
                    return r1[e]
                return r23[e][:, (qc - 2) * TQ:(qc - 1) * TQ]
            get.load_rest = load_rest
            return get

        def qkv_tiles(b):
            QT = qkvp.tile([128, T], BF16, tag="qk", name=f"QT{b}")
            KT = qkvp.tile([128, T], BF16, tag="qk", name=f"KT{b}")
            VT = vtp.tile([128, T], BF16, tag="vt", name=f"VT{b}")
            VS = vsp.tile([128, NKT, 2 * (D + 1)], BF16, tag="vs", name=f"VS{b}")
            ones_cols = VS.rearrange("p k (h x) -> p k h x", h=2)[:, :, :, D]
            nc.vector.tensor_copy(
                out=ones_cols, in_=onecol_f.broadcast_to([128, NKT, 2, 1])[:, :, :, 0]
            )
            return QT, KT, VT, VS

        def qkv_fillers(b, xts, QT, KT, VT, VS):
            """PE-heavy tasks for batch b's QKV + V-transpose, emitted one at
            a time inside another batch's ACT-bound attention loop."""
            tasks = []
            for fi, (dst, bias_t) in enumerate(((QT, bias_ts[0]), (KT, bias_ts[1]), (VT, bias_ts[2]))):
                for qc in range(NQC):
                    def chain(fi=fi, dst=dst, bias_t=bias_t, qc=qc):
                        ps = psum.tile([128, TQ], F32, tag="sm", bufs=2, name="qkvps")
                        for e in range(NET):
                            nc.tensor.matmul(
                                ps,
                                lhsT=wt_fi[fi][:, e, :],
                                rhs=xts(e, qc),
                                start=(e == 0),
                                stop=(e == NET - 1),
                            )
                        nc.vector.tensor_scalar_add(
                            out=dst[:, qc * TQ:(qc + 1) * TQ], in0=ps, scalar1=bias_t[:, 0:1]
                        )
                    tasks.append(chain)
            for kt in range(NKT):
                def vtr(kt=kt):
                    pt = psum.tile([128, 128], BF16, tag="sm", bufs=2, name="vtps")
                    nc.tensor.transpose(pt, VT[:, kt * 128:(kt + 1) * 128], ident)
                    out_ap = VS[:, kt, :].rearrange("p (h x) -> p h x", h=2)[:, :, 0:D]
                    in_ap = pt.rearrange("p (h d) -> p h d", h=2)
                    nc.vector.tensor_copy(out=out_ap, in_=in_ap)
                tasks.append(vtr)
            return tasks

        def proj_fillers(b, AO, tts):
            tasks = []
            for tt in tts:
                def pj(tt=tt):
                    ot = outp.tile([128, C], F32, tag="out", name="ot")
                    for cc in range(C // TQ):
                        pp = psum.tile([128, TQ], F32, tag="sm", bufs=2, name="projps")
                        nc.tensor.matmul(
                            pp,
                            lhsT=AO[:, tt * 128:(tt + 1) * 128],
                            rhs=wo_t[:, cc * TQ:(cc + 1) * TQ],
                            start=True,
                            stop=True,
                        )
                        nc.vector.tensor_copy(out=ot[:, cc * TQ:(cc + 1) * TQ], in_=pp)
                    nc.sync.dma_start(out=y[b, tt * 128:(tt + 1) * 128, :], in_=ot)
                tasks.append(pj)
            return tasks

        def attention(b, QT, KT, VS, fill_q, self_proj=False, fast_tail=False):
            """ACT-bound attention for batch b; pops PE filler tasks from
            fill_q each iteration to keep the TensorEngine saturated. With
            self_proj, finished q-chunks' projection tasks are appended to
            fill_q so only the last chunk's projection trails the loop."""
            AO = aop.tile([128, T], BF16, tag="ao", name=f"AO{b}")
            n_iter = NQC * (NKT // 2)
            it = 0

            def pop_fillers():
                remaining = n_iter - it
                if remaining <= 0 or not fill_q:
                    return
                k = -(-len(fill_q) // remaining)  # ceil
                for _ in range(min(k, len(fill_q))):
                    fill_q.popleft()()

            for qc in range(NQC):
                U = [psum.tile([D + 1, TQ], F32, tag="u", bufs=2, name=f"U{hh}") for hh in range(HPC)]
                pending_u = deque()
                for kt in range(NKT):
                    # both heads' scores land in one 2-bank PSUM tile; the
                    # K=64 matmuls hit disjoint PE row groups and disjoint
                    # banks, so they stream concurrently with no waits
                    # between them, and one exp covers both heads.
                    ssc = psum.tile([128, 2 * TQ], F32, tag="ss", bufs=2, name="ssc")
                    exc = expp.tile([128, 2 * TQ], BF16, tag="exp", name="exc")
                    for h in range(HPC):
                        nc.tensor.matmul(
                            ssc[:, h * TQ:(h + 1) * TQ],
                            lhsT=KT[h * D:(h + 1) * D, kt * 128:(kt + 1) * 128],
                            rhs=QT[h * D:(h + 1) * D, qc * TQ:(qc + 1) * TQ],
                            start=True,
                            stop=True,
                        )
                    nc.scalar.activation(out=exc, in_=ssc, func=AF.Exp, scale=SCALE)
                    # PE filler work + previous k-tile's attn@V run while the
                    # scalar engine computes this k-tile's exp
                    if kt % 2 == 0:
                        pop_fillers()
                        it += 1
                    if len(pending_u) >= 2:
                        pending_u.popleft()()
                    def make_u(kt=kt, exc=exc):
                        def emit_u():
                            for h in range(HPC):
                                nc.tensor.matmul(
                                    U[h],
                                    lhsT=VS[:, kt, h * (D + 1):(h + 1) * (D + 1)],
                                    rhs=exc[:, h * TQ:(h + 1) * TQ],
                                    start=(kt == 0),
                                    stop=(kt == NKT - 1),
                                )
                        return emit_u
                    pending_u.append(make_u())
                while pending_u:
                    pending_u.popleft()()
                last = fast_tail and qc == NQC - 1
                for h in range(HPC):
                    scp = smallp.tile([1, TQ], F32, tag="scp", name="scp")
                    nc.vector.tensor_copy(out=scp, in_=U[h][D:D + 1, :])
                    # NOTE: reciprocal_approx_fast NaNs on partition-
                    # shifted inputs; it must read the partition-0 copy.
                    rec = smallp.tile([1, TQ], F32, tag="rec", name="rec")
                    with nc.allow_low_precision(reason="softmax denom"):
                        nc.vector.reciprocal_approx_fast(out=rec, in_=scp)
                    rb = smallp.tile([D, TQ], F32, tag="rb", name="rb")
                    if last:
                        # latency-critical final chunk: broadcast 1/sum on the
                        # PE (ones[1,64].T @ rec) instead of the DRAM bounce
                        bcp = psum.tile([D, TQ], F32, tag="sm", bufs=2, name="bcp")
                        nc.tensor.matmul(bcp, lhsT=ones_row, rhs=rec, start=True, stop=True)
                        nc.vector.tensor_copy(out=rb, in_=bcp)
                        nc.vector.tensor_mul(
                            out=AO[h * D:(h + 1) * D, qc * TQ:(qc + 1) * TQ],
                            in0=U[h][0:D, :],
                            in1=rb,
                        )
                        continue
                    # release U fast: copy the numerator to SBUF, then
                    # partition-broadcast 1/sum via DRAM bounce
                    ucp = smallp.tile([D, TQ], F32, tag="ucp", name="ucp")
                    nc.vector.tensor_copy(out=ucp, in_=U[h][0:D, :])
                    nt = dramp.tile([1, TQ], F32, tag="nrm", name="nt")
                    nc.gpsimd.dma_start(out=nt, in_=rec)
                    nc.gpsimd.dma_start(out=rb, in_=nt.partition_broadcast(D))
                    nc.vector.tensor_mul(
                        out=AO[h * D:(h + 1) * D, qc * TQ:(qc + 1) * TQ],
                        in0=ucp,
                        in1=rb,
                    )
                if self_proj:
                    tts = range(qc * (TQ // 128), (qc + 1) * (TQ // 128))
                    fill_q.extend(proj_fillers(b, AO, tts))
            while fill_q:
                fill_q.popleft()()
            return AO

        # ---- software pipeline over batches ----
        xts0 = load_xt(0, defer_rest=True)
        nc.sync.dma_start(out=wt_k, in_=wk.rearrange("(e p) f -> p e f", p=128))
        nc.sync.dma_start(out=wt_v, in_=wv.rearrange("(e p) f -> p e f", p=128))
        xts0.load_rest()
        bias_ts = []
        for p in (bq, bk, bv):
            bt = consts.tile([FPC, 1], F32, tag="bias", bufs=3)
            nc.sync.dma_start(out=bt, in_=p[:, :])
            bias_ts.append(bt)
        wo_t = consts.tile([128, C], BF16)
        nc.sync.dma_start(out=wo_t, in_=wo[:, :])
        q0 = qkv_tiles(0)
        f0 = qkv_fillers(0, xts0, *q0)
        for t in f0:
            t()
        xts1 = load_xt(1)
        q1 = qkv_tiles(1)
        f1 = qkv_fillers(1, xts1, *q1)
        if PIPELINE:
            AO0 = attention(0, q0[0], q0[1], q0[3], deque(f1))
            p0 = deque(proj_fillers(0, AO0, range(T // 128)))
            attention(1, q1[0], q1[1], q1[3], p0, self_proj=True, fast_tail=True)
        else:
            AO0 = attention(0, q0[0], q0[1], q0[3], deque())
            for t in f1:
                t()
            for t in proj_fillers(0, AO0, range(T // 128)):
                t()
            AO1 = attention(1, q1[0], q1[1], q1[3], deque())
            for t in proj_fillers(1, AO1, range(T // 128)):
                t()

    nc.finalize()
    return nc


_NC_CACHE = None


def _get_nc():
    global _NC_CACHE
    if _NC_CACHE is None:
        _NC_CACHE = build_nc()
    return _NC_CACHE


def make_in_maps(x, W_qkv, b_qkv, W_out):
    xT = np.ascontiguousarray(x.transpose(0, 2, 1)).astype(NPBF16)
    Wb = W_qkv.astype(NPBF16)
    Wob = W_out.astype(NPBF16)
    in_maps = []
    for c in range(NCORES):
        f0 = c * FPC
        in_maps.append(
            {
                "xT": xT,
                "wq": np.ascontiguousarray(Wb[:, f0:f0 + FPC]),
                "wk": np.ascontiguousarray(Wb[:, C + f0:C + f0 + FPC]),
                "wv": np.ascontiguousarray(Wb[:, 2 * C + f0:2 * C + f0 + FPC]),
                "bq": np.ascontiguousarray(b_qkv[f0:f0 + FPC, None]).astype(np.float32),
                "bk": np.ascontiguousarray(b_qkv[C + f0:C + f0 + FPC, None]).astype(np.float32),
                "bv": np.ascontiguousarray(b_qkv[2 * C + f0:2 * C + f0 + FPC, None]).astype(np.float32),
                "wo": np.ascontiguousarray(Wob[f0:f0 + FPC, :]),
            }
        )
    return in_maps


def kernel(x, W_qkv, b_qkv, W_out, b_out, _trace=False, _trace_kwargs=None):
    x = np.asarray(x, dtype=np.float32)
    W_qkv = np.asarray(W_qkv, dtype=np.float32)
    b_qkv = np.asarray(b_qkv, dtype=np.float32)
    W_out = np.asarray(W_out, dtype=np.float32)
    b_out = np.asarray(b_out, dtype=np.float32)

    nc = _get_nc()
    in_maps = make_in_maps(x, W_qkv, b_qkv, W_out)
    res = run_bass_kernel_spmd(
        nc, in_maps, core_ids=list(range(NCORES)), trace=_trace,
        **(_trace_kwargs or {}),
    )
    y = res.results[0]["y"].astype(np.float64)
    for c in range(1, NCORES):
        y += res.results[c]["y"]
    y += b_out
    out = y.astype(np.float32)
    if _trace:
        return out, res
    return out
